# revision 8
# baseline (speedup 1.0000x reference)
"""Trainium2 Bass kernel for nn_BaseKernelSetConv (gnn_message_passing), v3.

Why not gather on device: HW probes show the SWDGE gather ucode costs
~8.7ns of Q7 time per gathered row per queue (~2.3ns/row across all 4
queues) -> the ~440k rows/core this op needs are a >1ms floor, and
indirect DMA ([128,1] form) is worse at ~8.7ns/row with no queue
parallelism (the v1 3.5ms bottleneck).

v3 therefore computes the score matrix DENSELY, data-parallel over
nodes with zero device-side gathers: for every node n, z[:, n] =
[unit(W)]_all @ x_n for all 14 weight groups (per degree d: focal
unit(W_focal_d) and slot s unit(W_nei_d[:,s])/d) = 224 scores.  Each
core streams its own shard (pre-transposed feature-major fp16 by the
host), runs two fp16 matmuls per 512 columns (224 = 128 + 96 output
rows), copies psum to fp16 staging, and writes z back.  The host then
assembles the output: per degree, score(n) = z[focal_d, n]/||x_n|| +
sum_s z[(d,s), nei(n,s)]/||x_nei||  (norm folding: z is computed on
raw x; dividing by the target row norm afterwards equals using
unit(x), exactly as the reference's CosineSimilarity).

Per-core HW work: 8MB shard read + 48MB z write + ~500 matmuls +
copies.  No indirect DMA, no Pool-engine work, no collectives.
"""

import sys
import numpy as np
import ml_dtypes

BF16 = ml_dtypes.bfloat16

sys.path.insert(0, "/opt/trn_rl_repo")

N = 1_000_000
F = 32
K = 16
NCORES = 8
SHARD = N // NCORES              # 125000 nodes per core
GRP = 512                        # columns per matmul group
NGRP = (SHARD + GRP - 1) // GRP  # 245 -> pad shard cols
COLS = NGRP * GRP                # 125440 padded columns per core
STG = 8                          # groups per staging tile / out DMA

_PROG = None


def _stream_list():
    out = []
    for d in (1, 2, 3, 4):
        out.append((d, -1))
        for s in range(d):
            out.append((d, s))
    return out


STREAMS = _stream_list()         # 14 streams x 16 kernels = 224 rows
NSTR = len(STREAMS)
NA = 8                           # streams in group A (128 psum rows)
NB = NSTR - NA                   # 6 streams in group B (96 rows)


def _unit_rows(a):
    a = a.astype(np.float64)
    return a / (np.linalg.norm(a, axis=-1, keepdims=True) + 1e-8)


def _build_program():
    import concourse.tile as tile
    from concourse import bacc, mybir

    f32 = mybir.dt.float32
    f16 = mybir.dt.bfloat16

    nc = bacc.Bacc("TRN2", target_bir_lowering=False, debug=False,
                   num_devices=NCORES)
    xT_d = nc.dram_tensor("xT", (F, COLS), f16, kind="ExternalInput").ap()
    wa_d = nc.dram_tensor("wa", (F, NA * K), f16, kind="ExternalInput").ap()
    wb_d = nc.dram_tensor("wb", (F, NB * K), f16, kind="ExternalInput").ap()
    za_d = nc.dram_tensor("za", (NA * K, COLS), f16,
                          kind="ExternalOutput").ap()
    zb_d = nc.dram_tensor("zb", (NB * K, COLS), f16,
                          kind="ExternalOutput").ap()

    TW = STG * GRP               # staging width (4096 cols)

    with tile.TileContext(nc) as tc:
        with tc.tile_pool(name="wp", bufs=1) as wp, \
             tc.tile_pool(name="xp", bufs=3) as xp, \
             tc.tile_pool(name="sa", bufs=2) as sa_p, \
             tc.tile_pool(name="sb", bufs=2) as sb_p, \
             tc.tile_pool(name="pa", bufs=2, space="PSUM") as pa_p, \
             tc.tile_pool(name="pb", bufs=2, space="PSUM") as pb_p:

            wa = wp.tile([F, NA * K], f16, tag="wa")
            nc.sync.dma_start(wa[:], wa_d[:])
            wb = wp.tile([F, NB * K], f16, tag="wb")
            nc.sync.dma_start(wb[:], wb_d[:])

            for blk in range(0, NGRP, STG):
                nst = min(STG, NGRP - blk)
                w = nst * GRP
                xt = xp.tile([F, TW], f16, tag="xt", name="xt")
                nc.sync.dma_start(
                    xt[:, :w], xT_d[:, blk * GRP:blk * GRP + w])
                stga = sa_p.tile([NA * K, TW], f16, tag="sa", name="sa")
                stgb = sb_p.tile([NB * K, TW], f16, tag="sb", name="sb")
                def cp(i, out, in_):
                    # gpsimd has no PSUM access; split psum drains DVE/ACT
                    if i % 2 == 0:
                        nc.vector.tensor_copy(out, in_)
                    else:
                        nc.scalar.copy(out, in_)
                # all A-group matmuls back-to-back (one weight set), then
                # all B -- fewer PE weight swaps, denser MM stream
                for g in range(nst):
                    rhs = xt[:, g * GRP:(g + 1) * GRP]
                    pa = pa_p.tile([NA * K, GRP], f32, tag="pa", name="pa")
                    nc.tensor.matmul(pa[:], lhsT=wa[:], rhs=rhs,
                                     start=True, stop=True)
                    cp(g, stga[:, g * GRP:(g + 1) * GRP], pa[:])
                for g in range(nst):
                    rhs = xt[:, g * GRP:(g + 1) * GRP]
                    pb = pb_p.tile([NB * K, GRP], f32, tag="pb", name="pb")
                    nc.tensor.matmul(pb[:], lhsT=wb[:], rhs=rhs,
                                     start=True, stop=True)
                    cp(g + 1, stgb[:, g * GRP:(g + 1) * GRP], pb[:])
                eng = nc.sync if (blk // STG) % 2 == 0 else nc.scalar
                eng.dma_start(za_d[:, blk * GRP:blk * GRP + w], stga[:, :w])
                eng = nc.scalar if (blk // STG) % 2 == 0 else nc.sync
                eng.dma_start(zb_d[:, blk * GRP:blk * GRP + w], stgb[:, :w])

    nc.compile()
    return nc


def host_prep(inputs):
    x = np.asarray(inputs["x"], dtype=np.float32)
    norm = np.sqrt((x.astype(np.float64) ** 2).sum(-1)) + 1e-8
    invn = (1.0 / norm).astype(np.float32)

    # weights: (32, 16) lhsT per stream, fp16
    wts = []
    for d, s in STREAMS:
        if s < 0:
            w = _unit_rows(np.asarray(inputs[f"W_focal{d}"], np.float32))
        else:
            wn = np.asarray(inputs[f"W_nei{d}"], np.float32)
            w = _unit_rows(wn.reshape(-1, F)).reshape(K, d, F)[:, s, :] / d
        wts.append(w.T.astype(BF16))          # (32, 16)
    wa = np.ascontiguousarray(np.concatenate(wts[:NA], axis=1))   # (32,128)
    wb = np.ascontiguousarray(np.concatenate(wts[NA:], axis=1))   # (32,96)

    xh = x.astype(BF16)
    in_maps = []
    for c in range(NCORES):
        sh = np.zeros((COLS, F), BF16)
        sh[:SHARD] = xh[c * SHARD:(c + 1) * SHARD]
        in_maps.append({"xT": np.ascontiguousarray(sh.T),
                        "wa": wa, "wb": wb})
    return in_maps, invn


def assemble(results, inputs, invn):
    za = np.concatenate([results[c]["za"] for c in range(NCORES)], axis=1)
    zb = np.concatenate([results[c]["zb"] for c in range(NCORES)], axis=1)
    z = np.concatenate([za, zb], axis=0)            # (224, 8*COLS) f16

    def col(n):
        return COLS * (n // SHARD) + (n % SHARD)

    t_of = {ds: t for t, ds in enumerate(STREAMS)}
    res = np.zeros((N, 64), np.float32)
    for d in (1, 2, 3, 4):
        nodes = np.asarray(inputs[f"selected_index_deg{d}"]).astype(np.int64)
        nei = np.asarray(inputs[f"nei_index_deg{d}"]).astype(np.int64) \
            .reshape(-1, d)
        t = t_of[(d, -1)]
        acc = (z[16 * t:16 * t + 16, col(nodes)].T.astype(np.float32)
               * invn[nodes][:, None])
        for s in range(d):
            t = t_of[(d, s)]
            tg = nei[:, s]
            acc += (z[16 * t:16 * t + 16, col(tg)].T.astype(np.float32)
                    * invn[tg][:, None])
        res[nodes, 16 * (d - 1):16 * d] = acc
    return res


LAST_RESULTS = None


def kernel(**inputs):
    global _PROG, LAST_RESULTS
    import os
    from concourse.bass_utils import run_bass_kernel_spmd
    in_maps, invn = host_prep(inputs)
    if _PROG is None:
        _PROG = _build_program()
    trace = bool(os.environ.get("BKC_TRACE"))
    res = run_bass_kernel_spmd(_PROG, in_maps, core_ids=list(range(NCORES)),
                               trace=trace)
    LAST_RESULTS = res
    return assemble(res.results, inputs, invn)


# ---------------------------------------------------------------------------
def kernel_emulated(**inputs):
    in_maps, invn = host_prep(inputs)
    results = []
    for m in in_maps:
        xT = m["xT"].astype(np.float32)
        za = (m["wa"].astype(np.float32).T @ xT).astype(BF16)
        zb = (m["wb"].astype(np.float32).T @ xT).astype(BF16)
        results.append({"za": za, "zb": zb})
    return assemble(results, inputs, invn)


# revision 9
# speedup vs baseline: 1.2195x; 1.2195x over previous
"""Trainium2 Bass kernel for nn_BaseKernelSetConv (gnn_message_passing), v3.

Why not gather on device: HW probes show the SWDGE gather ucode costs
~8.7ns of Q7 time per gathered row per queue (~2.3ns/row across all 4
queues) -> the ~440k rows/core this op needs are a >1ms floor, and
indirect DMA ([128,1] form) is worse at ~8.7ns/row with no queue
parallelism (the v1 3.5ms bottleneck).

v3 therefore computes the score matrix DENSELY, data-parallel over
nodes with zero device-side gathers: for every node n, z[:, n] =
[unit(W)]_all @ x_n for all 14 weight groups (per degree d: focal
unit(W_focal_d) and slot s unit(W_nei_d[:,s])/d) = 224 scores.  Each
core streams its own shard (pre-transposed feature-major fp16 by the
host), runs two fp16 matmuls per 512 columns (224 = 128 + 96 output
rows), copies psum to fp16 staging, and writes z back.  The host then
assembles the output: per degree, score(n) = z[focal_d, n]/||x_n|| +
sum_s z[(d,s), nei(n,s)]/||x_nei||  (norm folding: z is computed on
raw x; dividing by the target row norm afterwards equals using
unit(x), exactly as the reference's CosineSimilarity).

Per-core HW work: 8MB shard read + 48MB z write + ~500 matmuls +
copies.  No indirect DMA, no Pool-engine work, no collectives.
"""

import sys
import numpy as np

sys.path.insert(0, "/opt/trn_rl_repo")

N = 1_000_000
F = 32
K = 16
NCORES = 8
SHARD = N // NCORES              # 125000 nodes per core
GRP = 512                        # columns per matmul group
NGRP = (SHARD + GRP - 1) // GRP  # 245 -> pad shard cols
COLS = NGRP * GRP                # 125440 padded columns per core
STG = 8                          # groups per staging tile / out DMA

_PROG = None


def _stream_list():
    out = []
    for d in (1, 2, 3, 4):
        out.append((d, -1))
        for s in range(d):
            out.append((d, s))
    return out


STREAMS = _stream_list()         # 14 streams x 16 kernels = 224 rows
NSTR = len(STREAMS)
NA = 8                           # streams in group A (128 psum rows)
NB = NSTR - NA                   # 6 streams in group B (96 rows)


def _unit_rows(a):
    a = a.astype(np.float64)
    return a / (np.linalg.norm(a, axis=-1, keepdims=True) + 1e-8)


def _build_program():
    import concourse.tile as tile
    from concourse import bacc, mybir

    f32 = mybir.dt.float32
    f16 = mybir.dt.float16

    nc = bacc.Bacc("TRN2", target_bir_lowering=False, debug=False,
                   num_devices=NCORES)
    xT_d = nc.dram_tensor("xT", (F, COLS), f16, kind="ExternalInput").ap()
    wa_d = nc.dram_tensor("wa", (F, NA * K), f16, kind="ExternalInput").ap()
    wb_d = nc.dram_tensor("wb", (F, NB * K), f16, kind="ExternalInput").ap()
    za_d = nc.dram_tensor("za", (NA * K, COLS), f16,
                          kind="ExternalOutput").ap()
    zb_d = nc.dram_tensor("zb", (NB * K, COLS), f16,
                          kind="ExternalOutput").ap()

    TW = STG * GRP               # staging width (4096 cols)

    with tile.TileContext(nc) as tc:
        with tc.tile_pool(name="wp", bufs=1) as wp, \
             tc.tile_pool(name="xp", bufs=3) as xp, \
             tc.tile_pool(name="sa", bufs=2) as sa_p, \
             tc.tile_pool(name="sb", bufs=2) as sb_p, \
             tc.tile_pool(name="pa", bufs=4, space="PSUM") as pa_p, \
             tc.tile_pool(name="pb", bufs=4, space="PSUM") as pb_p:

            wa = wp.tile([F, NA * K], f16, tag="wa")
            nc.sync.dma_start(wa[:], wa_d[:])
            wb = wp.tile([F, NB * K], f16, tag="wb")
            nc.sync.dma_start(wb[:], wb_d[:])

            for blk in range(0, NGRP, STG):
                nst = min(STG, NGRP - blk)
                w = nst * GRP
                xt = xp.tile([F, TW], f16, tag="xt", name="xt")
                nc.sync.dma_start(
                    xt[:, :w], xT_d[:, blk * GRP:blk * GRP + w])
                stga = sa_p.tile([NA * K, TW], f16, tag="sa", name="sa")
                stgb = sb_p.tile([NB * K, TW], f16, tag="sb", name="sb")
                for g in range(nst):
                    rhs = xt[:, g * GRP:(g + 1) * GRP]
                    pa = pa_p.tile([NA * K, GRP], f32, tag="pa", name="pa")
                    nc.tensor.matmul(pa[:], lhsT=wa[:], rhs=rhs,
                                     start=True, stop=True)
                    nc.any.tensor_copy(stga[:, g * GRP:(g + 1) * GRP], pa[:])
                    pb = pb_p.tile([NB * K, GRP], f32, tag="pb", name="pb")
                    nc.tensor.matmul(pb[:], lhsT=wb[:], rhs=rhs,
                                     start=True, stop=True)
                    nc.any.tensor_copy(stgb[:, g * GRP:(g + 1) * GRP], pb[:])
                eng = nc.sync if (blk // STG) % 2 == 0 else nc.scalar
                eng.dma_start(za_d[:, blk * GRP:blk * GRP + w], stga[:, :w])
                eng = nc.scalar if (blk // STG) % 2 == 0 else nc.sync
                eng.dma_start(zb_d[:, blk * GRP:blk * GRP + w], stgb[:, :w])

    nc.compile()
    return nc


def host_prep(inputs):
    x = np.asarray(inputs["x"], dtype=np.float32)
    norm = np.sqrt((x.astype(np.float64) ** 2).sum(-1)) + 1e-8
    invn = (1.0 / norm).astype(np.float32)

    # weights: (32, 16) lhsT per stream, fp16
    wts = []
    for d, s in STREAMS:
        if s < 0:
            w = _unit_rows(np.asarray(inputs[f"W_focal{d}"], np.float32))
        else:
            wn = np.asarray(inputs[f"W_nei{d}"], np.float32)
            w = _unit_rows(wn.reshape(-1, F)).reshape(K, d, F)[:, s, :] / d
        wts.append(w.T.astype(np.float16))          # (32, 16)
    wa = np.ascontiguousarray(np.concatenate(wts[:NA], axis=1))   # (32,128)
    wb = np.ascontiguousarray(np.concatenate(wts[NA:], axis=1))   # (32,96)

    xh = x.astype(np.float16)
    in_maps = []
    for c in range(NCORES):
        sh = np.zeros((COLS, F), np.float16)
        sh[:SHARD] = xh[c * SHARD:(c + 1) * SHARD]
        in_maps.append({"xT": np.ascontiguousarray(sh.T),
                        "wa": wa, "wb": wb})
    return in_maps, invn


def assemble(results, inputs, invn):
    za = np.concatenate([results[c]["za"] for c in range(NCORES)], axis=1)
    zb = np.concatenate([results[c]["zb"] for c in range(NCORES)], axis=1)
    z = np.concatenate([za, zb], axis=0)            # (224, 8*COLS) f16

    def col(n):
        return COLS * (n // SHARD) + (n % SHARD)

    t_of = {ds: t for t, ds in enumerate(STREAMS)}
    res = np.zeros((N, 64), np.float32)
    for d in (1, 2, 3, 4):
        nodes = np.asarray(inputs[f"selected_index_deg{d}"]).astype(np.int64)
        nei = np.asarray(inputs[f"nei_index_deg{d}"]).astype(np.int64) \
            .reshape(-1, d)
        t = t_of[(d, -1)]
        acc = (z[16 * t:16 * t + 16, col(nodes)].T.astype(np.float32)
               * invn[nodes][:, None])
        for s in range(d):
            t = t_of[(d, s)]
            tg = nei[:, s]
            acc += (z[16 * t:16 * t + 16, col(tg)].T.astype(np.float32)
                    * invn[tg][:, None])
        res[nodes, 16 * (d - 1):16 * d] = acc
    return res


LAST_RESULTS = None


def kernel(**inputs):
    global _PROG, LAST_RESULTS
    import os
    from concourse.bass_utils import run_bass_kernel_spmd
    in_maps, invn = host_prep(inputs)
    if _PROG is None:
        _PROG = _build_program()
    trace = bool(os.environ.get("BKC_TRACE"))
    res = run_bass_kernel_spmd(_PROG, in_maps, core_ids=list(range(NCORES)),
                               trace=trace)
    LAST_RESULTS = res
    return assemble(res.results, inputs, invn)


# ---------------------------------------------------------------------------
def kernel_emulated(**inputs):
    in_maps, invn = host_prep(inputs)
    results = []
    for m in in_maps:
        xT = m["xT"].astype(np.float32)
        za = (m["wa"].astype(np.float32).T @ xT).astype(np.float16)
        zb = (m["wb"].astype(np.float32).T @ xT).astype(np.float16)
        results.append({"za": za, "zb": zb})
    return assemble(results, inputs, invn)


# revision 10
# speedup vs baseline: 1.3789x; 1.1307x over previous
"""v4: like v3 (dense z, host aggregation) but with 4-node packed columns.

rhs columns carry FOUR nodes' features stacked (128-row contraction);
weights are 4-band block-diagonal [128,128] tiles, one per 32-row group
of z (7 groups).  Column streams drop from 2*COLS to 1.75*COLS and the
128-row contraction gives the PE's activity monitor a denser signal
(v3's 32-row matmuls never unthrottled the PE from 1.2 to 2.4 GHz).

Output zq_g[32a+j, m] = z[32g+j, node 4m+a]; the host de-interleaves.
"""

import sys
import numpy as np
import ml_dtypes

BF16 = ml_dtypes.bfloat16

sys.path.insert(0, "/opt/trn_rl_repo")

N = 1_000_000
F = 32
K = 16
NCORES = 8
SHARD = N // NCORES              # 125000
GRP = 512                        # packed columns per matmul (2048 nodes)
NODES_PER_COL = 4
COLS4 = 31360                    # packed columns per core (>= 31250), 512-mult
NGRP = COLS4 // GRP              # 61.25 -> pad: 61.25?? must divide
STG = 2                          # groups per staging tile
NG = 7                           # 32-row z groups (224 rows)

assert COLS4 % GRP == 0 or True

_PROG = None


def _stream_list():
    out = []
    for d in (1, 2, 3, 4):
        out.append((d, -1))
        for s in range(d):
            out.append((d, s))
    return out


STREAMS = _stream_list()
NSTR = len(STREAMS)              # 14


def _unit_rows(a):
    a = a.astype(np.float64)
    return a / (np.linalg.norm(a, axis=-1, keepdims=True) + 1e-8)


def _build_program():
    import concourse.tile as tile
    from concourse import bacc, mybir

    f32 = mybir.dt.float32
    bf = mybir.dt.bfloat16

    ngrp = (COLS4 + GRP - 1) // GRP
    cols = ngrp * GRP

    nc = bacc.Bacc("TRN2", target_bir_lowering=False, debug=False,
                   num_devices=NCORES)
    xq_d = nc.dram_tensor("xq", (128, cols), bf, kind="ExternalInput").ap()
    w_d = nc.dram_tensor("w", (128, NG * 128), bf, kind="ExternalInput").ap()
    zq_d = nc.dram_tensor("zq", (128, NG * cols), bf,
                          kind="ExternalOutput").ap()

    TW = STG * GRP

    with tile.TileContext(nc) as tc:
        with tc.tile_pool(name="wp", bufs=1) as wp, \
             tc.tile_pool(name="xp", bufs=3) as xp, \
             tc.tile_pool(name="sp", bufs=2) as sp, \
             tc.tile_pool(name="pp", bufs=4, space="PSUM") as pp:

            wt = wp.tile([128, NG * 128], bf, tag="wt")
            nc.sync.dma_start(wt[:], w_d[:])

            for blk in range(0, ngrp, STG):
                nst = min(STG, ngrp - blk)
                w = nst * GRP
                xt = xp.tile([128, TW], bf, tag="xt", name="xt")
                nc.sync.dma_start(
                    xt[:, :w], xq_d[:, blk * GRP:blk * GRP + w])
                stgs = [sp.tile([128, TW], bf, tag=f"s{gi}", name=f"s{gi}")
                        for gi in range(NG)]
                for g in range(nst):
                    rhs = xt[:, g * GRP:(g + 1) * GRP]
                    for gi in range(NG):
                        pt = pp.tile([128, GRP], f32, tag="pt", name="pt")
                        nc.tensor.matmul(
                            pt[:], lhsT=wt[:, gi * 128:(gi + 1) * 128],
                            rhs=rhs, start=True, stop=True)
                        if (g * NG + gi) % 2 == 0:
                            nc.vector.tensor_copy(
                                stgs[gi][:, g * GRP:(g + 1) * GRP], pt[:])
                        else:
                            nc.scalar.copy(
                                stgs[gi][:, g * GRP:(g + 1) * GRP], pt[:])
                for gi in range(NG):
                    eng = nc.sync if gi % 2 == 0 else nc.scalar
                    eng.dma_start(
                        zq_d[:, gi * cols + blk * GRP:
                             gi * cols + blk * GRP + w],
                        stgs[gi][:, :w])

    nc.compile()
    return nc


def host_prep(inputs):
    x = np.asarray(inputs["x"], dtype=np.float32)
    norm = np.sqrt((x.astype(np.float64) ** 2).sum(-1)) + 1e-8
    invn = (1.0 / norm).astype(np.float32)

    wts = []
    for d, s in STREAMS:
        if s < 0:
            w = _unit_rows(np.asarray(inputs[f"W_focal{d}"], np.float32))
        else:
            wn = np.asarray(inputs[f"W_nei{d}"], np.float32)
            w = _unit_rows(wn.reshape(-1, F)).reshape(K, d, F)[:, s, :] / d
        wts.append(w.T.astype(np.float32))           # (32, 16)
    wall = np.concatenate(wts, axis=1)               # (32, 224)

    # 7 block-diag tiles: wt_g[32a+f, 32a+j] = wall[f, 32g+j]
    wtile = np.zeros((128, NG * 128), np.float32)
    for g in range(NG):
        blkw = wall[:, 32 * g:32 * g + 32]           # (32, 32)
        for a in range(4):
            wtile[32 * a:32 * a + 32,
                  g * 128 + 32 * a:g * 128 + 32 * a + 32] = blkw
    wtile = wtile.astype(BF16)

    ngrp = (COLS4 + GRP - 1) // GRP
    cols = ngrp * GRP
    xh = x.astype(BF16)
    in_maps = []
    for c in range(NCORES):
        sh = np.zeros((cols * 4, F), BF16)
        sh[:SHARD] = xh[c * SHARD:(c + 1) * SHARD]
        # xq[32a+f, m] = x[4m+a, f]
        xq = np.ascontiguousarray(
            sh.reshape(cols, 4, F).transpose(1, 2, 0).reshape(128, cols))
        in_maps.append({"xq": xq, "w": wtile})
    return in_maps, invn, cols


def assemble(results, inputs, invn, cols):
    # zq per core: (128, NG*cols); z[32g+j, node 4m+a] = zq[32a+j, g*cols+m]
    zq = np.concatenate([results[c]["zq"] for c in range(NCORES)], axis=1)
    # reshape to [4, 32, NCORES, NG, cols]
    zq = zq.reshape(4, 32, NCORES, NG, cols)

    t_of = {ds: t for t, ds in enumerate(STREAMS)}

    def lookup(t, tgt):
        """z rows [16t,16t+16) for target nodes -> (n,16) f32"""
        g, j0 = (16 * t) // 32, (16 * t) % 32
        core = tgt // SHARD
        local = tgt % SHARD
        a = local % 4
        m = local // 4
        block = zq[:, j0:j0 + 16, :, g, :]           # (4, 16, NCORES, cols)
        return block[a[:, None], np.arange(16)[None, :],
                     core[:, None], m[:, None]].astype(np.float32)

    res = np.zeros((N, 64), np.float32)
    for d in (1, 2, 3, 4):
        nodes = np.asarray(inputs[f"selected_index_deg{d}"]).astype(np.int64)
        nei = np.asarray(inputs[f"nei_index_deg{d}"]).astype(np.int64) \
            .reshape(-1, d)
        acc = lookup(t_of[(d, -1)], nodes) * invn[nodes][:, None]
        for s in range(d):
            tg = nei[:, s]
            acc += lookup(t_of[(d, s)], tg) * invn[tg][:, None]
        res[nodes, 16 * (d - 1):16 * d] = acc
    return res


LAST_RESULTS = None


def kernel(**inputs):
    global _PROG, LAST_RESULTS
    import os
    from concourse.bass_utils import run_bass_kernel_spmd
    in_maps, invn, cols = host_prep(inputs)
    if _PROG is None:
        _PROG = _build_program()
    trace = bool(os.environ.get("BKC_TRACE"))
    res = run_bass_kernel_spmd(_PROG, in_maps, core_ids=list(range(NCORES)),
                               trace=trace)
    LAST_RESULTS = res
    return assemble(res.results, inputs, invn, cols)


def kernel_emulated(**inputs):
    in_maps, invn, cols = host_prep(inputs)
    results = []
    for m in in_maps:
        xq = m["xq"].astype(np.float32)              # (128, cols)
        wt = m["w"].astype(np.float32)               # (128, NG*128)
        zq = np.empty((128, NG * cols), np.float32)
        for g in range(NG):
            zq[:, g * cols:(g + 1) * cols] = \
                wt[:, g * 128:(g + 1) * 128].T @ xq
        results.append({"zq": zq.astype(BF16)})
    return assemble(results, inputs, invn, cols)


# revision 11
# speedup vs baseline: 1.4234x; 1.0322x over previous
"""v5: v4 + copy/DMA-issue diet.

v4 trace: Vector 107% / Scalar 99% busy -- 434 single-bank psum->bf16
copies (689ns each), 381 sem-waits on Vector, and 249 out-DMA issues.
v5 pairs matmuls into 2-bank [128,1024] f32 psum tiles (half the
copies, each twice as wide => less per-instr overhead + fewer sems)
and interleaves the z DRAM layout by (column-group, z-group) so each
STG-block flushes with ONE large DMA on Sync (31 out-DMAs total).

zq layout: (128, ngrp*NG*GRP); element [32a+j, (gcol*NG+gi)*GRP+e] =
z[32gi+j, node 4*(gcol*GRP+e)+a].
"""

import sys
import numpy as np
import ml_dtypes

BF16 = ml_dtypes.bfloat16

sys.path.insert(0, "/opt/trn_rl_repo")

N = 1_000_000
F = 32
K = 16
NCORES = 8
SHARD = N // NCORES              # 125000
GRP = 512                        # packed columns per matmul (2048 nodes)
COLS4 = 31250                    # packed columns needed per core
NG = 7                           # 32-row z groups (224 rows)
STG = 2                          # column-groups per staging block

_PROG = None


def _stream_list():
    out = []
    for d in (1, 2, 3, 4):
        out.append((d, -1))
        for s in range(d):
            out.append((d, s))
    return out


STREAMS = _stream_list()
NSTR = len(STREAMS)


def _unit_rows(a):
    a = a.astype(np.float64)
    return a / (np.linalg.norm(a, axis=-1, keepdims=True) + 1e-8)


def _geom():
    ngrp = (COLS4 + GRP - 1) // GRP          # 62
    cols = ngrp * GRP                        # 31744
    return ngrp, cols


def _build_program():
    import concourse.tile as tile
    from concourse import bacc, mybir

    f32 = mybir.dt.float32
    bf = mybir.dt.bfloat16
    ngrp, cols = _geom()

    nc = bacc.Bacc("TRN2", target_bir_lowering=False, debug=False,
                   num_devices=NCORES)
    xq_d = nc.dram_tensor("xq", (128, cols), bf, kind="ExternalInput").ap()
    w_d = nc.dram_tensor("w", (128, NG * 128), bf, kind="ExternalInput").ap()
    zq_d = nc.dram_tensor("zq", (128, ngrp * NG * GRP), bf,
                          kind="ExternalOutput").ap()

    with tile.TileContext(nc) as tc:
        with tc.tile_pool(name="wp", bufs=1) as wp, \
             tc.tile_pool(name="xp", bufs=3) as xp, \
             tc.tile_pool(name="sp", bufs=2) as sp, \
             tc.tile_pool(name="pp", bufs=4, space="PSUM") as pp:

            wt = wp.tile([128, NG * 128], bf, tag="wt")
            nc.sync.dma_start(wt[:], w_d[:])

            for blk in range(0, ngrp, STG):
                nst = min(STG, ngrp - blk)
                bw = nst * NG * GRP
                xt = xp.tile([128, STG * GRP], bf, tag="xt", name="xt")
                nc.sync.dma_start(
                    xt[:, :nst * GRP],
                    xq_d[:, blk * GRP:(blk + nst) * GRP])
                stg = sp.tile([128, STG * NG * GRP], bf, tag="st", name="st")
                seq = [(g, gi) for g in range(nst) for gi in range(NG)]
                for p in range(0, len(seq), 2):
                    pair = seq[p:p + 2]
                    pt = pp.tile([128, 2 * GRP], f32, tag="pt", name="pt")
                    for h, (g, gi) in enumerate(pair):
                        nc.tensor.matmul(
                            pt[:, h * GRP:(h + 1) * GRP],
                            lhsT=wt[:, gi * 128:(gi + 1) * 128],
                            rhs=xt[:, g * GRP:(g + 1) * GRP],
                            start=True, stop=True)
                    dst = stg[:, p * GRP:(p + len(pair)) * GRP]
                    src = pt[:, :len(pair) * GRP]
                    if (p // 2) % 2 == 0:
                        nc.vector.tensor_copy(dst, src)
                    else:
                        nc.scalar.copy(dst, src)
                nc.sync.dma_start(
                    zq_d[:, blk * NG * GRP:blk * NG * GRP + bw],
                    stg[:, :bw])

    nc.compile()
    return nc


def host_prep(inputs):
    x = np.asarray(inputs["x"], dtype=np.float32)
    norm = np.sqrt((x.astype(np.float64) ** 2).sum(-1)) + 1e-8
    invn = (1.0 / norm).astype(np.float32)

    wts = []
    for d, s in STREAMS:
        if s < 0:
            w = _unit_rows(np.asarray(inputs[f"W_focal{d}"], np.float32))
        else:
            wn = np.asarray(inputs[f"W_nei{d}"], np.float32)
            w = _unit_rows(wn.reshape(-1, F)).reshape(K, d, F)[:, s, :] / d
        wts.append(w.T.astype(np.float32))
    wall = np.concatenate(wts, axis=1)               # (32, 224)

    wtile = np.zeros((128, NG * 128), np.float32)
    for g in range(NG):
        blkw = wall[:, 32 * g:32 * g + 32]
        for a in range(4):
            wtile[32 * a:32 * a + 32,
                  g * 128 + 32 * a:g * 128 + 32 * a + 32] = blkw
    wtile = wtile.astype(BF16)

    ngrp, cols = _geom()
    xh = x.astype(BF16)
    in_maps = []
    for c in range(NCORES):
        sh = np.zeros((cols * 4, F), BF16)
        sh[:SHARD] = xh[c * SHARD:(c + 1) * SHARD]
        xq = np.ascontiguousarray(
            sh.reshape(cols, 4, F).transpose(1, 2, 0).reshape(128, cols))
        in_maps.append({"xq": xq, "w": wtile})
    return in_maps, invn


def assemble(results, inputs, invn):
    ngrp, cols = _geom()
    zq = np.concatenate([results[c]["zq"] for c in range(NCORES)], axis=1)
    # (4, 32, NCORES, ngrp, NG, GRP)
    zq = zq.reshape(4, 32, NCORES, ngrp, NG, GRP)

    t_of = {ds: t for t, ds in enumerate(STREAMS)}
    k16 = np.arange(16)[None, :]

    def lookup(t, tgt):
        gi, j0 = (16 * t) // 32, (16 * t) % 32
        core = tgt // SHARD
        local = tgt % SHARD
        a = local % 4
        m = local // 4
        gcol = m // GRP
        e = m % GRP
        blk = zq[:, j0:j0 + 16, :, :, gi, :]         # (4,16,NCORES,ngrp,GRP)
        return blk[a[:, None], k16, core[:, None], gcol[:, None],
                   e[:, None]].astype(np.float32)

    res = np.zeros((N, 64), np.float32)
    for d in (1, 2, 3, 4):
        nodes = np.asarray(inputs[f"selected_index_deg{d}"]).astype(np.int64)
        nei = np.asarray(inputs[f"nei_index_deg{d}"]).astype(np.int64) \
            .reshape(-1, d)
        acc = lookup(t_of[(d, -1)], nodes) * invn[nodes][:, None]
        for s in range(d):
            tg = nei[:, s]
            acc += lookup(t_of[(d, s)], tg) * invn[tg][:, None]
        res[nodes, 16 * (d - 1):16 * d] = acc
    return res


LAST_RESULTS = None


def kernel(**inputs):
    global _PROG, LAST_RESULTS
    import os
    from concourse.bass_utils import run_bass_kernel_spmd
    in_maps, invn = host_prep(inputs)
    if _PROG is None:
        _PROG = _build_program()
    trace = bool(os.environ.get("BKC_TRACE"))
    res = run_bass_kernel_spmd(_PROG, in_maps, core_ids=list(range(NCORES)),
                               trace=trace)
    LAST_RESULTS = res
    return assemble(res.results, inputs, invn)


def kernel_emulated(**inputs):
    in_maps, invn = host_prep(inputs)
    ngrp, cols = _geom()
    results = []
    for m in in_maps:
        xq = m["xq"].astype(np.float32)              # (128, cols)
        wt = m["w"].astype(np.float32)
        zq5 = np.empty((128, ngrp, NG, GRP), np.float32)
        for gi in range(NG):
            zlin = wt[:, gi * 128:(gi + 1) * 128].T @ xq   # (128, cols)
            zq5[:, :, gi, :] = zlin.reshape(128, ngrp, GRP)
        results.append({"zq": zq5.reshape(128, -1).astype(BF16)})
    return assemble(results, inputs, invn)
